# revision 51
# baseline (speedup 1.0000x reference)
"""Fused QKV projection (dense transformer attention prologue) on 8 TRN2 NeuronCores.

Reference computation:
    qkv = hidden_states @ concat([Wq, Wk, Wv], axis=1) + concat([bq, bk, bv])
    q, k, v = split(qkv) -> each reshaped to [B, H, S, D] = [4, 16, 4096, 64]

Strategy: data-parallel over tokens (B*S = 16384 tokens -> 2048 per core),
which minimizes per-core HBM traffic (x slice 8 MiB + replicated W 12 MiB +
y slice 24 MiB = 44 MiB/core) vs head-sharded tensor parallelism (~90 MiB).

Each core computes y^T[f, tok] = W^T x^T + b for its token slice:
  - W (fp32 in DRAM) is cast fp32->bf16 inline by the SWDGE DMA load, in
    [128, 384] column chunks so the first matmuls can start early.
  - x is transposed on-chip with PE identity transposes (fp32 in, PSUM out)
    and cast to bf16 by the DVE eviction. Transposes for token groups 1..3
    are emitted lazily inside phase 0 so they overlap the matmul stream.
  - Matmuls run in bf16 with fp32 PSUM accumulation (K=1024 = 8 k-tiles,
    N=512 = one PSUM bank); y^T orientation puts the fused output features
    on partitions, so the bias lands as a per-partition scalar.
  - The bias add is fused into the PSUM eviction (DVE tensor_scalar_add),
    costing nothing extra; y streams out in [128, 512] chunks.
Host side only shards / concatenates / reassembles layouts.

Cost-model exec time ~185 us/core; validated on HW via work-scaling slope
(~150-175 us per repeated GEMM phase vs ~165 us modeled).
"""

import numpy as np

import concourse.bass as bass
import concourse.mybir as mybir
from concourse import bacc
from concourse.bass import ds, ts
from concourse.bass_utils import run_bass_kernel_spmd
from concourse.masks import make_identity
from concourse.tile import TileContext

# Problem shapes (hardcoded per contract; kernel.py must be self-contained).
B, S = 4, 4096
HID = 1024
NH, HD = 16, 64
F = 3 * HID              # 3072 fused output features
NCORES = 8
TOK = B * S              # 16384
TOK_PC = TOK // NCORES   # 2048 tokens per core

P = 128
KT = HID // P            # 8 k tiles
XT = TOK_PC // P         # 16 x token tiles
NG = TOK_PC // 512       # 4 token groups of 512 (matmul N)
FT = F // P              # 24 f-tiles total
FH = 384                 # W column chunk (f per DMA)
NH_W = F // FH           # 8 W column chunks
FTH = FH // P            # 3 f-tiles per W chunk

FP32 = mybir.dt.float32
BF16 = mybir.dt.bfloat16


def _build_nc(repeat: int = 1) -> bass.Bass:
    # Bacc (not raw Bass): its compile() runs move_matmul_waits_to_ldweights /
    # generate_event_semaphores, which walrus needs (1 sync-wait per inst).
    # `repeat` replays the main GEMM phase (benchmark-only work scaling).
    nc = bacc.Bacc("TRN2")
    x = nc.declare_dram_parameter("x", [TOK_PC, HID], FP32, isOutput=False)
    w = nc.declare_dram_parameter("w", [HID, F], FP32, isOutput=False)
    bvec = nc.declare_dram_parameter("bvec", [F], FP32, isOutput=False)
    y = nc.declare_dram_parameter("y", [F, TOK_PC], FP32, isOutput=True)

    with TileContext(nc) as tc:
        with (
            tc.tile_pool(name="const", bufs=1) as const_pool,
            tc.tile_pool(name="xin", bufs=XT) as x_pool,
            tc.tile_pool(name="xtp", bufs=KT * NG) as xt_pool,
            tc.tile_pool(name="wsb", bufs=KT * NH_W) as w_pool,
            tc.tile_pool(name="ysb", bufs=8) as y_pool,
            tc.tile_pool(name="pstr", bufs=4, space="PSUM") as pstr_pool,
            tc.tile_pool(name="psmm", bufs=4, space="PSUM") as psmm_pool,
        ):
            # --- constants -------------------------------------------------
            ident = const_pool.tile([P, P], FP32, name="ident")
            make_identity(nc, ident)

            # bias laid out [partition, f_tile]: bias_sb[p, f] = bvec[f*128+p].
            # One contiguous [24, 128] DMA, then a PE transpose (K=24) into
            # PSUM and a DVE copy — lands in ~2us instead of 24 tiny DMAs.
            bias_rows = const_pool.tile([FT, P], FP32, name="bias_rows")
            nc.scalar.dma_start(
                out=bias_rows, in_=bvec.rearrange("(f p) -> f p", p=P)
            )
            bias_sb = const_pool.tile([P, FT], FP32, name="bias_sb")
            ps_b = pstr_pool.tile([P, 512], FP32, name="ps_bias", tag="pstr")
            nc.tensor.transpose(ps_b[:, :FT], bias_rows, ident[:FT, :FT])
            nc.vector.tensor_copy(bias_sb, ps_b[:, :FT])

            # --- input DMAs ------------------------------------------------
            # x token tiles [128, 1024] fp32, alternating the two HWDGE rings
            # (SP / ACT) so the early tiles land ~2x sooner than one FIFO.
            def _x_dma(t):
                xt = x_pool.tile([P, HID], FP32, name=f"x{t}", tag="x")
                eng = nc.sync if t % 2 == 0 else nc.scalar
                eng.dma_start(out=xt, in_=x[ts(t, P), :])
                return xt

            x_tiles = [_x_dma(t) for t in range(4)]

            # W tiles per (k, column-chunk): [128, 384] bf16, cast fp32->bf16
            # inline (SWDGE). First chunk (f 0:384, all 8 k) ships first so
            # f=0..2 matmuls can start early.
            w_half = {}

            def _w_dma(k, h):
                wt = w_pool.tile([P, FH], BF16, name=f"w{k}h{h}", tag="w")
                nc.gpsimd.dma_start(out=wt, in_=w[ts(k, P), ds(h * FH, FH)])
                w_half[(k, h)] = wt

            for k in range(KT):
                _w_dma(k, 0)

            x_tiles += [_x_dma(t) for t in range(4, XT)]

            for h in range(1, NH_W):
                for k in range(KT):
                    _w_dma(k, h)

            # --- x transpose ----------------------------------------------
            # xT tile (k, g) holds x^T[k*128:(k+1)*128, g*512:(g+1)*512] bf16.
            xT = {}

            def _transpose_group(g, x_major=False):
                # x_major: iterate source tiles outermost (half the k range
                # at a time so only 4 pstr banks are open) — the PE never
                # stalls waiting for the later x tiles of the group.
                ps_of, bf_of = {}, {}
                for k in range(KT):
                    bf_of[k] = xt_pool.tile(
                        [P, 512], BF16, name=f"xT{g}_{k}", tag="xT"
                    )
                k_batches = (
                    [range(0, 4), range(4, 8)] if x_major else [range(KT)]
                )
                for ks in k_batches:
                    for k in ks:
                        ps_of[k] = pstr_pool.tile(
                            [P, 512], FP32, name=f"ps{g}_{k}", tag="pstr"
                        )
                    if x_major:
                        for i in range(4):
                            for k in ks:
                                nc.tensor.transpose(
                                    ps_of[k][:, ts(i, P)],
                                    x_tiles[4 * g + i][:, ts(k, P)],
                                    ident,
                                )
                    else:
                        for k in ks:
                            for i in range(4):
                                nc.tensor.transpose(
                                    ps_of[k][:, ts(i, P)],
                                    x_tiles[4 * g + i][:, ts(k, P)],
                                    ident,
                                )
                    for k in ks:
                        nc.vector.tensor_copy(bf_of[k], ps_of[k])
                for k in range(KT):
                    xT[(k, g)] = bf_of[k]

            # Group 0 up front (x-major so it starts as soon as x0 lands);
            # groups 1..3 are emitted lazily inside phase 0 so their PSUM
            # evictions interleave with the y evictions on the DVE FIFO.
            _transpose_group(0, x_major=True)
            lazy_pts = {3: 1, 9: 2, 15: 3}

            # --- main GEMM + fused bias + store ----------------------------
            # token-group-outer: phase g sweeps all 24 f-tiles for one group
            # of 512 tokens; xT for group g is only needed at phase g, so the
            # later transposes hide inside phase 0's matmul stream.
            for rep in range(repeat):
                for g in range(NG):
                    for f in range(FT):
                        acc = psmm_pool.tile(
                            [P, 512], FP32, name=f"acc{g}_{f}", tag="acc"
                        )
                        for k in range(KT):
                            nc.tensor.matmul(
                                acc,
                                w_half[(k, f // FTH)][:, ts(f % FTH, P)],
                                xT[(k, g)],
                                start=(k == 0),
                                stop=(k == KT - 1),
                            )
                        # PSUM -> SBUF eviction with fused per-partition bias,
                        # then the [128, 512] chunk streams straight out.
                        ych = y_pool.tile([P, 512], FP32, name=f"y{g}_{f}", tag="y")
                        nc.vector.tensor_scalar_add(
                            ych, acc, bias_sb[:, f : f + 1]
                        )
                        nc.scalar.dma_start(
                            out=y[ts(f, P), ds(g * 512, 512)], in_=ych
                        )
                        if rep == 0 and g == 0 and f in lazy_pts:
                            _transpose_group(lazy_pts[f])

    nc.finalize()  # runs Bacc.compile(): reg alloc + sync-wait legalization
    return nc


_NC_CACHE = {}

# test-harness hooks: set TRACE=True before calling kernel() to profile the
# run; the full BassKernelResults lands in LAST_RESULTS either way.
TRACE = False
LAST_RESULTS = None


def _get_nc(repeat: int = 1) -> bass.Bass:
    if repeat not in _NC_CACHE:
        _NC_CACHE[repeat] = _build_nc(repeat)
    return _NC_CACHE[repeat]


def kernel(hidden_states, Wq, bq, Wk, bk, Wv, bv):
    hidden_states = np.asarray(hidden_states, dtype=np.float32)
    w = np.concatenate(
        [np.asarray(Wq, np.float32), np.asarray(Wk, np.float32), np.asarray(Wv, np.float32)],
        axis=1,
    )
    bvec = np.concatenate(
        [np.asarray(bq, np.float32), np.asarray(bk, np.float32), np.asarray(bv, np.float32)]
    )

    x = np.ascontiguousarray(hidden_states.reshape(TOK, HID))
    in_maps = [
        {"x": x[c * TOK_PC : (c + 1) * TOK_PC], "w": w, "bvec": bvec}
        for c in range(NCORES)
    ]

    nc = _get_nc()
    res = run_bass_kernel_spmd(nc, in_maps, list(range(NCORES)), trace=TRACE)
    global LAST_RESULTS
    LAST_RESULTS = res
    outs = res.results

    q = np.empty((B, NH, S, HD), np.float32)
    k = np.empty((B, NH, S, HD), np.float32)
    v = np.empty((B, NH, S, HD), np.float32)
    for c in range(NCORES):
        yT = np.asarray(outs[c]["y"])             # [3072, 2048]
        part = yT.reshape(3, NH, HD, TOK_PC)      # [qkv, h, d, tok]
        b_i, s_i = divmod(c, S // TOK_PC)
        s0 = s_i * TOK_PC
        q[b_i, :, s0 : s0 + TOK_PC, :] = part[0].transpose(0, 2, 1)
        k[b_i, :, s0 : s0 + TOK_PC, :] = part[1].transpose(0, 2, 1)
        v[b_i, :, s0 : s0 + TOK_PC, :] = part[2].transpose(0, 2, 1)
    return q, k, v


# revision 55
# speedup vs baseline: 1.0237x; 1.0237x over previous
"""Fused QKV projection (dense transformer attention prologue) on 8 TRN2 NeuronCores.

Reference computation:
    qkv = hidden_states @ concat([Wq, Wk, Wv], axis=1) + concat([bq, bk, bv])
    q, k, v = split(qkv) -> each reshaped to [B, H, S, D] = [4, 16, 4096, 64]

Strategy: data-parallel over tokens (B*S = 16384 tokens -> 2048 per core),
which minimizes per-core HBM traffic (x slice 8 MiB + replicated W 12 MiB +
y slice 24 MiB = 44 MiB/core) vs head-sharded tensor parallelism (~90 MiB).

Each core computes y^T[f, tok] = W^T x^T + b for its token slice:
  - W (fp32 in DRAM) is cast fp32->bf16 inline by the SWDGE DMA load, in
    [128, 384] column chunks so the first matmuls can start early.
  - x is transposed on-chip with PE identity transposes (fp32 in, PSUM out)
    and cast to bf16 by the DVE eviction. Transposes for token groups 1..3
    are emitted lazily inside phase 0 so they overlap the matmul stream.
  - Matmuls run in bf16 with fp32 PSUM accumulation (K=1024 = 8 k-tiles,
    N=512 = one PSUM bank); y^T orientation puts the fused output features
    on partitions, so the bias lands as a per-partition scalar.
  - The bias add is fused into the PSUM eviction (DVE tensor_scalar_add),
    costing nothing extra; y streams out in [128, 512] chunks.
Host side only shards / concatenates / reassembles layouts.

Cost-model exec time ~185 us/core; validated on HW via work-scaling slope
(~150-175 us per repeated GEMM phase vs ~165 us modeled).
"""

import numpy as np

import concourse.bass as bass
import concourse.mybir as mybir
from concourse import bacc
from concourse.bass import ds, ts
from concourse.bass_utils import run_bass_kernel_spmd
from concourse.masks import make_identity
from concourse.tile import TileContext

# Problem shapes (hardcoded per contract; kernel.py must be self-contained).
B, S = 4, 4096
HID = 1024
NH, HD = 16, 64
F = 3 * HID              # 3072 fused output features
NCORES = 8
TOK = B * S              # 16384
TOK_PC = TOK // NCORES   # 2048 tokens per core

P = 128
KT = HID // P            # 8 k tiles
XT = TOK_PC // P         # 16 x token tiles
NG = TOK_PC // 512       # 4 token groups of 512 (matmul N)
FT = F // P              # 24 f-tiles total
FH = 384                 # W column chunk (f per DMA)
NH_W = F // FH           # 8 W column chunks
FTH = FH // P            # 3 f-tiles per W chunk

FP32 = mybir.dt.float32
BF16 = mybir.dt.bfloat16


def _build_nc(repeat: int = 1) -> bass.Bass:
    # Bacc (not raw Bass): its compile() runs move_matmul_waits_to_ldweights /
    # generate_event_semaphores, which walrus needs (1 sync-wait per inst).
    # `repeat` replays the main GEMM phase (benchmark-only work scaling).
    nc = bacc.Bacc("TRN2")
    x = nc.declare_dram_parameter("x", [TOK_PC, HID], FP32, isOutput=False)
    w = nc.declare_dram_parameter("w", [HID, F], FP32, isOutput=False)
    bvec = nc.declare_dram_parameter("bvec", [F], FP32, isOutput=False)
    y = nc.declare_dram_parameter("y", [F, TOK_PC], FP32, isOutput=True)

    with TileContext(nc) as tc:
        with (
            tc.tile_pool(name="const", bufs=1) as const_pool,
            tc.tile_pool(name="xin", bufs=6) as x_pool,
            tc.tile_pool(name="xbf", bufs=XT) as xbf_pool,
            tc.tile_pool(name="xtp", bufs=KT * NG) as xt_pool,
            tc.tile_pool(name="wsb", bufs=KT * NH_W) as w_pool,
            tc.tile_pool(name="ysb", bufs=8) as y_pool,
            tc.tile_pool(name="pstr", bufs=2, space="PSUM") as pstr_pool,
            tc.tile_pool(name="psmm", bufs=6, space="PSUM") as psmm_pool,
        ):
            # --- constants -------------------------------------------------
            ident = const_pool.tile([P, P], FP32, name="ident")
            make_identity(nc, ident)
            # bf16 identity for the x transposes: a bf16 transpose streams at
            # 1 cycle/row on the PE vs 2 for fp32 — halves the transpose cost
            identb = const_pool.tile([P, P], BF16, name="identb")
            make_identity(nc, identb)

            # bias laid out [partition, f_tile]: bias_sb[p, f] = bvec[f*128+p].
            # One contiguous [24, 128] DMA, then a PE transpose (K=24) into
            # PSUM and a DVE copy — lands in ~2us instead of 24 tiny DMAs.
            bias_rows = const_pool.tile([FT, P], FP32, name="bias_rows")
            nc.scalar.dma_start(
                out=bias_rows, in_=bvec.rearrange("(f p) -> f p", p=P)
            )
            bias_sb = const_pool.tile([P, FT], FP32, name="bias_sb")
            ps_b = pstr_pool.tile([P, 512], FP32, name="ps_bias", tag="pstr")
            nc.tensor.transpose(ps_b[:, :FT], bias_rows, ident[:FT, :FT])
            nc.vector.tensor_copy(bias_sb, ps_b[:, :FT])

            # --- input DMAs ------------------------------------------------
            # x token tiles [128, 1024] fp32, alternating the two HWDGE rings
            # (SP / ACT) so the early tiles land ~2x sooner than one FIFO.
            # fp32 load (HWDGE, alternating rings), then a DVE cast to bf16.
            # The fp32 staging tile is released right after the cast; the PE
            # transposes read the bf16 copy at half the fp32 streaming cost.
            def _x_dma(t):
                xt = x_pool.tile([P, HID], FP32, name=f"x{t}", tag="x")
                eng = nc.sync if t % 2 == 0 else nc.scalar
                eng.dma_start(out=xt, in_=x[ts(t, P), :])
                xb = xbf_pool.tile([P, HID], BF16, name=f"xb{t}", tag="xb")
                nc.vector.tensor_copy(xb, xt)
                return xb

            x_tiles = [_x_dma(t) for t in range(4)]

            # W tiles per (k, column-chunk): [128, 384] bf16, cast fp32->bf16
            # inline (SWDGE). First chunk (f 0:384, all 8 k) ships first so
            # f=0..2 matmuls can start early.
            w_half = {}

            def _w_dma(k, h):
                wt = w_pool.tile([P, FH], BF16, name=f"w{k}h{h}", tag="w")
                nc.gpsimd.dma_start(out=wt, in_=w[ts(k, P), ds(h * FH, FH)])
                w_half[(k, h)] = wt

            for k in range(KT):
                _w_dma(k, 0)

            x_tiles += [_x_dma(t) for t in range(4, XT)]

            for h in range(1, NH_W):
                for k in range(KT):
                    _w_dma(k, h)

            # --- x transpose ----------------------------------------------
            # xT tile (k, g) holds x^T[k*128:(k+1)*128, g*512:(g+1)*512] bf16.
            xT = {}

            def _transpose_group(g, x_major=False):
                # x_major: iterate source tiles outermost (half the k range
                # at a time so only 4 pstr banks are open) — the PE never
                # stalls waiting for the later x tiles of the group.
                ps_of, bf_of = {}, {}
                for k in range(KT):
                    bf_of[k] = xt_pool.tile(
                        [P, 512], BF16, name=f"xT{g}_{k}", tag="xT"
                    )
                k_batches = (
                    [range(0, 4), range(4, 8)] if x_major else [range(KT)]
                )
                for ks in k_batches:
                    for k in ks:
                        ps_of[k] = pstr_pool.tile(
                            [P, 512], BF16, name=f"ps{g}_{k}", tag="pstr"
                        )
                    if x_major:
                        for i in range(4):
                            for k in ks:
                                nc.tensor.transpose(
                                    ps_of[k][:, ts(i, P)],
                                    x_tiles[4 * g + i][:, ts(k, P)],
                                    identb,
                                )
                    else:
                        for k in ks:
                            for i in range(4):
                                nc.tensor.transpose(
                                    ps_of[k][:, ts(i, P)],
                                    x_tiles[4 * g + i][:, ts(k, P)],
                                    identb,
                                )
                    for k in ks:
                        nc.vector.tensor_copy(bf_of[k], ps_of[k])
                for k in range(KT):
                    xT[(k, g)] = bf_of[k]

            # Group 0 up front (x-major so it starts as soon as x0 lands);
            # groups 1..3 are emitted lazily inside phase 0 so their PSUM
            # evictions interleave with the y evictions on the DVE FIFO.
            _transpose_group(0, x_major=True)
            lazy_pts = {3: 1, 9: 2, 15: 3}

            # --- main GEMM + fused bias + store ----------------------------
            # token-group-outer: phase g sweeps all 24 f-tiles for one group
            # of 512 tokens; xT for group g is only needed at phase g, so the
            # later transposes hide inside phase 0's matmul stream.
            for rep in range(repeat):
                for g in range(NG):
                    for f in range(FT):
                        acc = psmm_pool.tile(
                            [P, 512], FP32, name=f"acc{g}_{f}", tag="acc"
                        )
                        for k in range(KT):
                            nc.tensor.matmul(
                                acc,
                                w_half[(k, f // FTH)][:, ts(f % FTH, P)],
                                xT[(k, g)],
                                start=(k == 0),
                                stop=(k == KT - 1),
                            )
                        # PSUM -> SBUF eviction with fused per-partition bias,
                        # then the [128, 512] chunk streams straight out.
                        ych = y_pool.tile([P, 512], FP32, name=f"y{g}_{f}", tag="y")
                        nc.vector.tensor_scalar_add(
                            ych, acc, bias_sb[:, f : f + 1]
                        )
                        nc.scalar.dma_start(
                            out=y[ts(f, P), ds(g * 512, 512)], in_=ych
                        )
                        if rep == 0 and g == 0 and f in lazy_pts:
                            _transpose_group(lazy_pts[f])

    nc.finalize()  # runs Bacc.compile(): reg alloc + sync-wait legalization
    return nc


_NC_CACHE = {}

# test-harness hooks: set TRACE=True before calling kernel() to profile the
# run; the full BassKernelResults lands in LAST_RESULTS either way.
TRACE = False
LAST_RESULTS = None


def _get_nc(repeat: int = 1) -> bass.Bass:
    if repeat not in _NC_CACHE:
        _NC_CACHE[repeat] = _build_nc(repeat)
    return _NC_CACHE[repeat]


def kernel(hidden_states, Wq, bq, Wk, bk, Wv, bv):
    hidden_states = np.asarray(hidden_states, dtype=np.float32)
    w = np.concatenate(
        [np.asarray(Wq, np.float32), np.asarray(Wk, np.float32), np.asarray(Wv, np.float32)],
        axis=1,
    )
    bvec = np.concatenate(
        [np.asarray(bq, np.float32), np.asarray(bk, np.float32), np.asarray(bv, np.float32)]
    )

    x = np.ascontiguousarray(hidden_states.reshape(TOK, HID))
    in_maps = [
        {"x": x[c * TOK_PC : (c + 1) * TOK_PC], "w": w, "bvec": bvec}
        for c in range(NCORES)
    ]

    nc = _get_nc()
    res = run_bass_kernel_spmd(nc, in_maps, list(range(NCORES)), trace=TRACE)
    global LAST_RESULTS
    LAST_RESULTS = res
    outs = res.results

    q = np.empty((B, NH, S, HD), np.float32)
    k = np.empty((B, NH, S, HD), np.float32)
    v = np.empty((B, NH, S, HD), np.float32)
    for c in range(NCORES):
        yT = np.asarray(outs[c]["y"])             # [3072, 2048]
        part = yT.reshape(3, NH, HD, TOK_PC)      # [qkv, h, d, tok]
        b_i, s_i = divmod(c, S // TOK_PC)
        s0 = s_i * TOK_PC
        q[b_i, :, s0 : s0 + TOK_PC, :] = part[0].transpose(0, 2, 1)
        k[b_i, :, s0 : s0 + TOK_PC, :] = part[1].transpose(0, 2, 1)
        v[b_i, :, s0 : s0 + TOK_PC, :] = part[2].transpose(0, 2, 1)
    return q, k, v


# revision 60
# speedup vs baseline: 1.0295x; 1.0056x over previous
"""Fused QKV projection (dense transformer attention prologue) on 8 TRN2 NeuronCores.

Reference computation:
    qkv = hidden_states @ concat([Wq, Wk, Wv], axis=1) + concat([bq, bk, bv])
    q, k, v = split(qkv) -> each reshaped to [B, H, S, D] = [4, 16, 4096, 64]

Strategy: data-parallel over tokens (B*S = 16384 tokens -> 2048 per core),
which minimizes per-core HBM traffic (x slice 8 MiB + replicated W 12 MiB +
y slice 24 MiB = 44 MiB/core) vs head-sharded tensor parallelism (~90 MiB).

Each core computes y^T[f, tok] = W^T x^T + b for its token slice:
  - W (fp32 in DRAM) is cast fp32->bf16 inline by the SWDGE DMA load, in
    [128, 384] column chunks so the first matmuls can start early.
  - x loads fp32 on the two HWDGE rings, is cast to bf16 by a DVE copy,
    then transposed with PE identity transposes in bf16 (1 cycle/row vs 2
    for fp32 — halves the transpose cost on the critical engine).
    Transposes for token groups 1..3 are emitted lazily inside phase 0 so
    they hide in the matmul stream.
  - Matmuls run in bf16 with fp32 PSUM accumulation (K=1024 = 8 k-tiles,
    N=512 = one PSUM bank); y^T orientation puts the fused output features
    on partitions, so the bias lands as a per-partition scalar.
  - The bias add is fused into the PSUM eviction (DVE tensor_scalar_add),
    costing nothing extra; y streams out in [128, 512] chunks.
Host side only shards / concatenates / reassembles layouts.

Cost-model exec time ~179 us/core (PE busy 170 us = 96% occupancy, zero
mid-kernel gaps >300ns); validated on HW via work-scaling slope (~150-196
us per repeated GEMM phase across runs vs ~165 us modeled).
"""

import numpy as np

import concourse.bass as bass
import concourse.mybir as mybir
from concourse import bacc
from concourse.bass import ds, ts
from concourse.bass_utils import run_bass_kernel_spmd
from concourse.masks import make_identity
from concourse.tile import TileContext

# Problem shapes (hardcoded per contract; kernel.py must be self-contained).
B, S = 4, 4096
HID = 1024
NH, HD = 16, 64
F = 3 * HID              # 3072 fused output features
NCORES = 8
TOK = B * S              # 16384
TOK_PC = TOK // NCORES   # 2048 tokens per core

P = 128
KT = HID // P            # 8 k tiles
XT = TOK_PC // P         # 16 x token tiles
NG = TOK_PC // 512       # 4 token groups of 512 (matmul N)
FT = F // P              # 24 f-tiles total
FH = 384                 # W column chunk (f per DMA)
NH_W = F // FH           # 8 W column chunks
FTH = FH // P            # 3 f-tiles per W chunk

FP32 = mybir.dt.float32
BF16 = mybir.dt.bfloat16


def _build_nc(repeat: int = 1) -> bass.Bass:
    # Bacc (not raw Bass): its compile() runs move_matmul_waits_to_ldweights /
    # generate_event_semaphores, which walrus needs (1 sync-wait per inst).
    # `repeat` replays the main GEMM phase (benchmark-only work scaling).
    nc = bacc.Bacc("TRN2")
    x = nc.declare_dram_parameter("x", [TOK_PC, HID], FP32, isOutput=False)
    w = nc.declare_dram_parameter("w", [HID, F], FP32, isOutput=False)
    bvec = nc.declare_dram_parameter("bvec", [F], FP32, isOutput=False)
    y = nc.declare_dram_parameter("y", [F, TOK_PC], FP32, isOutput=True)

    with TileContext(nc) as tc:
        with (
            tc.tile_pool(name="const", bufs=1) as const_pool,
            tc.tile_pool(name="xin", bufs=6) as x_pool,
            tc.tile_pool(name="xbf", bufs=XT) as xbf_pool,
            tc.tile_pool(name="xtp", bufs=KT * NG) as xt_pool,
            tc.tile_pool(name="wsb", bufs=KT * NH_W) as w_pool,
            tc.tile_pool(name="ysb", bufs=8) as y_pool,
            tc.tile_pool(name="pstr", bufs=2, space="PSUM") as pstr_pool,
            tc.tile_pool(name="psmm", bufs=6, space="PSUM") as psmm_pool,
        ):
            # --- constants -------------------------------------------------
            ident = const_pool.tile([P, P], FP32, name="ident")
            make_identity(nc, ident)
            # bf16 identity for the x transposes: a bf16 transpose streams at
            # 1 cycle/row on the PE vs 2 for fp32 — halves the transpose cost
            identb = const_pool.tile([P, P], BF16, name="identb")
            make_identity(nc, identb)

            # bias laid out [partition, f_tile]: bias_sb[p, f] = bvec[f*128+p].
            # One contiguous [24, 128] DMA, then a PE transpose (K=24) into
            # PSUM and a DVE copy — lands in ~2us instead of 24 tiny DMAs.
            bias_rows = const_pool.tile([FT, P], FP32, name="bias_rows")
            nc.scalar.dma_start(
                out=bias_rows, in_=bvec.rearrange("(f p) -> f p", p=P)
            )
            bias_sb = const_pool.tile([P, FT], FP32, name="bias_sb")
            ps_b = pstr_pool.tile([P, 512], FP32, name="ps_bias", tag="pstr")
            nc.tensor.transpose(ps_b[:, :FT], bias_rows, ident[:FT, :FT])
            nc.vector.tensor_copy(bias_sb, ps_b[:, :FT])

            # --- input DMAs ------------------------------------------------
            # x token tiles [128, 1024] fp32, alternating the two HWDGE rings
            # (SP / ACT) so the early tiles land ~2x sooner than one FIFO.
            # fp32 load (HWDGE, alternating rings), then a DVE cast to bf16.
            # The fp32 staging tile is released right after the cast; the PE
            # transposes read the bf16 copy at half the fp32 streaming cost.
            def _x_dma(t):
                xt = x_pool.tile([P, HID], FP32, name=f"x{t}", tag="x")
                xb = xbf_pool.tile([P, HID], BF16, name=f"xb{t}", tag="xb")
                eng = nc.sync if t % 2 == 0 else nc.scalar
                eng.dma_start(out=xt, in_=x[ts(t, P), :])
                nc.vector.tensor_copy(xb, xt)
                return xb

            # Token group 0 with half-tile granularity, all column-half-0
            # DMAs first (spread over both HWDGE rings): the x-major
            # transpose batches consume k 0..3 (= half 0) of all four tiles
            # first, so the PE starts ~2us earlier.
            H2 = HID // 2
            x_tiles = []
            xg0_f32 = []
            for t in range(4):
                xg0_f32.append(x_pool.tile([P, HID], FP32, name=f"x{t}", tag="x"))
                x_tiles.append(xbf_pool.tile([P, HID], BF16, name=f"xb{t}", tag="xb"))
            for h in range(2):
                cols = ds(h * H2, H2)
                for t in range(4):
                    eng = nc.sync if (t + h) % 2 == 0 else nc.scalar
                    eng.dma_start(out=xg0_f32[t][:, cols], in_=x[ts(t, P), cols])
                for t in range(4):
                    nc.vector.tensor_copy(x_tiles[t][:, cols], xg0_f32[t][:, cols])


            # W tiles per (k, column-chunk): [128, 384] bf16, cast fp32->bf16
            # inline (SWDGE). First chunk (f 0:384, all 8 k) ships first so
            # f=0..2 matmuls can start early.
            w_half = {}

            def _w_dma(k, h):
                wt = w_pool.tile([P, FH], BF16, name=f"w{k}h{h}", tag="w")
                nc.gpsimd.dma_start(out=wt, in_=w[ts(k, P), ds(h * FH, FH)])
                w_half[(k, h)] = wt

            for k in range(KT):
                _w_dma(k, 0)

            x_tiles += [_x_dma(t) for t in range(4, XT)]

            for h in range(1, NH_W):
                for k in range(KT):
                    _w_dma(k, h)

            # --- x transpose ----------------------------------------------
            # xT tile (k, g) holds x^T[k*128:(k+1)*128, g*512:(g+1)*512] bf16.
            xT = {}

            def _transpose_group(g, x_major=False):
                # x_major: iterate source tiles outermost (half the k range
                # at a time so only 4 pstr banks are open) — the PE never
                # stalls waiting for the later x tiles of the group.
                ps_of, bf_of = {}, {}
                for k in range(KT):
                    bf_of[k] = xt_pool.tile(
                        [P, 512], BF16, name=f"xT{g}_{k}", tag="xT"
                    )
                k_batches = (
                    [range(0, 4), range(4, 8)] if x_major else [range(KT)]
                )
                for ks in k_batches:
                    for k in ks:
                        ps_of[k] = pstr_pool.tile(
                            [P, 512], BF16, name=f"ps{g}_{k}", tag="pstr"
                        )
                    if x_major:
                        for i in range(4):
                            for k in ks:
                                nc.tensor.transpose(
                                    ps_of[k][:, ts(i, P)],
                                    x_tiles[4 * g + i][:, ts(k, P)],
                                    identb,
                                )
                    else:
                        for k in ks:
                            for i in range(4):
                                nc.tensor.transpose(
                                    ps_of[k][:, ts(i, P)],
                                    x_tiles[4 * g + i][:, ts(k, P)],
                                    identb,
                                )
                    for k in ks:
                        nc.vector.tensor_copy(bf_of[k], ps_of[k])
                for k in range(KT):
                    xT[(k, g)] = bf_of[k]

            # Group 0 up front (x-major so it starts as soon as x0 lands);
            # groups 1..3 are emitted lazily inside phase 0 so their PSUM
            # evictions interleave with the y evictions on the DVE FIFO.
            _transpose_group(0, x_major=True)
            lazy_pts = {3: 1, 9: 2, 15: 3}

            # --- main GEMM + fused bias + store ----------------------------
            # token-group-outer: phase g sweeps all 24 f-tiles for one group
            # of 512 tokens; xT for group g is only needed at phase g, so the
            # later transposes hide inside phase 0's matmul stream.
            for rep in range(repeat):
                for g in range(NG):
                    for f in range(FT):
                        acc = psmm_pool.tile(
                            [P, 512], FP32, name=f"acc{g}_{f}", tag="acc"
                        )
                        for k in range(KT):
                            nc.tensor.matmul(
                                acc,
                                w_half[(k, f // FTH)][:, ts(f % FTH, P)],
                                xT[(k, g)],
                                start=(k == 0),
                                stop=(k == KT - 1),
                            )
                        # PSUM -> SBUF eviction with fused per-partition bias,
                        # then the [128, 512] chunk streams straight out.
                        ych = y_pool.tile([P, 512], FP32, name=f"y{g}_{f}", tag="y")
                        nc.vector.tensor_scalar_add(
                            ych, acc, bias_sb[:, f : f + 1]
                        )
                        nc.scalar.dma_start(
                            out=y[ts(f, P), ds(g * 512, 512)], in_=ych
                        )
                        if rep == 0 and g == 0 and f in lazy_pts:
                            _transpose_group(lazy_pts[f])

    nc.finalize()  # runs Bacc.compile(): reg alloc + sync-wait legalization
    return nc


_NC_CACHE = {}

# test-harness hooks: set TRACE=True before calling kernel() to profile the
# run; the full BassKernelResults lands in LAST_RESULTS either way.
TRACE = False
LAST_RESULTS = None


def _get_nc(repeat: int = 1) -> bass.Bass:
    if repeat not in _NC_CACHE:
        _NC_CACHE[repeat] = _build_nc(repeat)
    return _NC_CACHE[repeat]


def kernel(hidden_states, Wq, bq, Wk, bk, Wv, bv):
    hidden_states = np.asarray(hidden_states, dtype=np.float32)
    w = np.concatenate(
        [np.asarray(Wq, np.float32), np.asarray(Wk, np.float32), np.asarray(Wv, np.float32)],
        axis=1,
    )
    bvec = np.concatenate(
        [np.asarray(bq, np.float32), np.asarray(bk, np.float32), np.asarray(bv, np.float32)]
    )

    x = np.ascontiguousarray(hidden_states.reshape(TOK, HID))
    in_maps = [
        {"x": x[c * TOK_PC : (c + 1) * TOK_PC], "w": w, "bvec": bvec}
        for c in range(NCORES)
    ]

    nc = _get_nc()
    res = run_bass_kernel_spmd(nc, in_maps, list(range(NCORES)), trace=TRACE)
    global LAST_RESULTS
    LAST_RESULTS = res
    outs = res.results

    q = np.empty((B, NH, S, HD), np.float32)
    k = np.empty((B, NH, S, HD), np.float32)
    v = np.empty((B, NH, S, HD), np.float32)
    for c in range(NCORES):
        yT = np.asarray(outs[c]["y"])             # [3072, 2048]
        part = yT.reshape(3, NH, HD, TOK_PC)      # [qkv, h, d, tok]
        b_i, s_i = divmod(c, S // TOK_PC)
        s0 = s_i * TOK_PC
        q[b_i, :, s0 : s0 + TOK_PC, :] = part[0].transpose(0, 2, 1)
        k[b_i, :, s0 : s0 + TOK_PC, :] = part[1].transpose(0, 2, 1)
        v[b_i, :, s0 : s0 + TOK_PC, :] = part[2].transpose(0, 2, 1)
    return q, k, v
